# revision 57
# baseline (speedup 1.0000x reference)
"""GQA kernel for Trainium2, 8 NeuronCores.

Sharding: core c = b*4 + g handles batch b, kv-head g (4 query heads).
Host sums the 4 partial out-projections per batch.

Design notes (v3):
- All matmuls bf16 (fp32 PSUM). PE cost is free-dim rows x clock, and the
  clock p-state ramps to 2.4GHz only after ~3us of *continuous* PE busy,
  so the emission order never lets the PE idle: projection, attention and
  out-projection chunks are interleaved, and the out-projection of chunk
  qc is folded into the first head's kt-loop of chunk qc+1.
- DMA descriptors are generated per SBUF partition line, so all DRAM
  layouts are partition-major: x is host-blocked [sc][p][t][f] (16KB
  contiguous per partition per chunk), weights [p][t][..], and the output
  is written per s-tile with 4KB lines. Chunk loads are split along t so
  8 queues stream one chunk in parallel and the first matmul of a chain
  only waits for its own t-range.
- Attention kt order: two non-diagonal tiles first (their exp has no
  DVE mask-add on the critical path, hiding the QK->exp->PV latency at
  each head-loop start), then the 4 diagonal tiles (column-trimmed: tile
  j only touches q columns >= 128j, the rest is fully masked), then the
  remaining tiles. The first flushed PV/l matmul covers all 512 columns
  so PSUM start=True initializes the full accumulator.
- Softmax normalization without a DRAM round trip: l row-sums accumulate
  via ones-matmuls per kt; then l -> bf16 copy (ACT), K=1 matmul
  broadcast across partitions (PE), reciprocal_approx_fast (DVE, ~0.7us
  vs 3.3us for reciprocal), multiply into onrm.
"""

import sys

import numpy as np

for _p in ("/opt/trn_rl_repo",):
    if _p not in sys.path:
        sys.path.insert(0, _p)

import ml_dtypes

import concourse.mybir as mybir
from concourse import bacc
from concourse.bass_utils import run_bass_kernel_spmd
from concourse.masks import make_identity
from concourse.tile import TileContext

B, S, E = 2, 2048, 2048
H, HKV = 16, 4
D = E // H  # 128
G = H // HKV  # 4 query heads per kv head
GD = G * D  # 512
NCORES = B * HKV  # 8
SC = 512  # s/q chunk width (free dim of matmuls)
NSC = S // SC  # 4
NET = E // 128  # 16 e-tiles (contraction)
NKT = S // 128  # 16 k-tiles
NEC = E // SC  # 4 e-chunks for output
SCALE = 1.0 / float(np.sqrt(D))

F32 = mybir.dt.float32
BF16 = mybir.dt.bfloat16
AF = mybir.ActivationFunctionType
NPBF = np.dtype(ml_dtypes.bfloat16)


def build_nc():
    nc = bacc.Bacc()
    # x inputs: [sc][p][t][f] partition-major blocks -> [NSC*128, NET*SC]
    xq = nc.declare_dram_parameter("xq", [NSC * 128, NET * SC], BF16, isOutput=False)
    xk = nc.declare_dram_parameter("xk", [NSC * 128, NET * SC], BF16, isOutput=False)
    xv = nc.declare_dram_parameter("xv", [NSC * 128, NET * SC], BF16, isOutput=False)
    # weights partition-major: [p][t][..]
    wq = nc.declare_dram_parameter("wq", [128, NET * GD], BF16, isOutput=False)
    wk = nc.declare_dram_parameter("wk", [128, NET * D], BF16, isOutput=False)
    wv = nc.declare_dram_parameter("wv", [128, NET * D], BF16, isOutput=False)
    wo = nc.declare_dram_parameter("wo", [128, G * E], BF16, isOutput=False)
    # natural [S, E]: written per s-tile as [128, E] full-width rows
    out = nc.declare_dram_parameter("out", [S, E], BF16, isOutput=True)

    with TileContext(nc) as tc:
        with (
            tc.tile_pool(name="singles", bufs=1) as singles,
            tc.tile_pool(name="xc", bufs=1) as xcp,
            tc.tile_pool(name="pexp", bufs=7) as pexp,
            tc.tile_pool(name="small", bufs=2) as small,
            tc.tile_pool(name="ob", bufs=2) as obp,
            tc.tile_pool(name="acc", bufs=4, space="PSUM") as acc,
            tc.tile_pool(name="ops", bufs=2, space="PSUM") as ops,
            tc.tile_pool(name="lps", bufs=2, space="PSUM") as lps,
        ):
            # ---- constants / weights resident in SBUF ----
            wq_sb = singles.tile([128, NET, GD], BF16)  # 16KB/p
            wk_sb = singles.tile([128, NET, D], BF16)  # 4KB/p
            wv_sb = singles.tile([128, NET, D], BF16)  # 4KB/p
            wo_sb = singles.tile([128, G, E], BF16)  # 16KB/p
            mask_sb = singles.tile([128, 4, SC], F32)  # 8KB/p
            ident_f = singles.tile([128, 128], F32)
            ident = singles.tile([128, 128], BF16)
            ones_f = singles.tile([128, 128], F32)
            # l-matmul lhsT: [128,128] ones -> row-sum REPLICATED on all 128
            # output partitions (same row cost as a 1-wide lhsT, but the
            # weight load pipelines and no separate broadcast is needed)
            ones_m = singles.tile([128, 128], BF16)
            qT = singles.tile([128, G, S], BF16)  # 16KB/p
            kT = singles.tile([128, S], BF16)  # 4KB/p
            v_sb = singles.tile([128, NKT, D], BF16)  # 4KB/p
            onrm = singles.tile([128, G, S], BF16)  # 16KB/p

            make_identity(nc, ident_f)
            nc.scalar.activation(out=ident[:], in_=ident_f[:], func=AF.Copy)
            nc.vector.memset(ones_f, 1.0)
            nc.scalar.activation(out=ones_m[:], in_=ones_f[:], func=AF.Copy)
            # causal mask built on-device (gpsimd, idle at t=0): tile j is the
            # additive mask for k-rows 128j..128j+127 vs q-columns of a chunk:
            # keep 0 where q - p - 128j >= 0, else -1e9.
            nc.gpsimd.memset(mask_sb, 0.0)
            for j in range(4):
                nc.gpsimd.affine_select(
                    out=mask_sb[:, j, :], in_=mask_sb[:, j, :],
                    compare_op=mybir.AluOpType.is_ge, fill=-1e9,
                    base=-128 * j, channel_multiplier=-1, pattern=[[1, SC]],
                )

            def wsplit(dst, dram, width, nsplit):
                """Load [128, n, width] SBUF tile from partition-major DRAM."""
                ntiles = dst.shape[1]
                step = ntiles // nsplit
                for i in range(nsplit):
                    t0 = i * step
                    nc.sync.dma_start(
                        out=dst[:, t0 : t0 + step, :],
                        in_=dram[:, t0 * width : (t0 + step) * width],
                    )

            # PE p-state warmup: the clock ramps to 2.4GHz only after ~3us of
            # continuous busy, and the first x split lands at ~14us. These
            # dummy matmuls bridge the idle window so real work starts at
            # full clock.
            warm_ps = acc.tile([128, SC], F32, tag="acc", name="warm_ps")

            def pad(k):
                """k dummy matmuls: PE filler that holds the p-state clock up
                across a known dependency stall (idle resets the 2.4GHz ramp)."""
                for _ in range(k):
                    nc.tensor.matmul(
                        warm_ps[:, 0:128], lhsT=ident[:], rhs=ident[:],
                        start=True, stop=True, skip_group_check=True,
                    )

            pad(80)

            state = {}  # (qc, h) -> (o_ps, l_ps)

            def x_chunk(dram, sc, tag):
                """Stream one [128, NET, SC] x chunk, split along t."""
                # xq double-buffers: its ring-1 WAR (next chunk's DMA vs the
                # 4 head chains of the previous Q-proj) otherwise serializes
                # the stream late in the run.
                xsb = xcp.tile(
                    [128, NET, SC], BF16, tag=tag, name=tag,
                    bufs=2 if tag == "xq" else 1,
                )
                rows = slice(sc * 128, (sc + 1) * 128)
                # 4KB descriptor lines in steady state; finer first chunk so
                # the first projection chain starts earlier
                nsplit = 8 if sc == 0 else 4
                step = NET // nsplit
                for i in range(nsplit):
                    t0 = i * step
                    nc.sync.dma_start(
                        out=xsb[:, t0 : t0 + step, :],
                        in_=dram[rows, t0 * SC : (t0 + step) * SC],
                    )
                return xsb

            def prefetch_x(sc):
                return (
                    x_chunk(xq, sc, "xq"),
                    x_chunk(xk, sc, "xk"),
                    x_chunk(xv, sc, "xv"),
                )

            def proj_block(sc, pre=None):
                # Q first: its 13.6us of chains pace the chunk's K/V x stream
                ssl = slice(sc * SC, (sc + 1) * SC)
                xsb = pre[0] if pre and pre[0] is not None else x_chunk(xq, sc, "xq")
                for h in range(G):
                    ps = acc.tile([128, SC], F32, tag="acc", name="qps")
                    for t in range(NET):
                        nc.tensor.matmul(
                            ps[:], lhsT=wq_sb[:, t, h * D : (h + 1) * D],
                            rhs=xsb[:, t, :], start=(t == 0), stop=(t == NET - 1),
                        )
                    # fold softmax scale into Q
                    nc.scalar.activation(
                        out=qT[:, h, ssl], in_=ps[:], func=AF.Copy, scale=SCALE
                    )
                    if sc == 0 and h == 0:
                        wsplit(wk_sb, wk, D, 4)
                        wsplit(wv_sb, wv, D, 4)
                # K projection
                xsb = pre[1] if pre and pre[1] is not None else x_chunk(xk, sc, "xk")
                ps = acc.tile([128, SC], F32, tag="acc", name="kps")
                for t in range(NET):
                    nc.tensor.matmul(
                        ps[:], lhsT=wk_sb[:, t, :], rhs=xsb[:, t, :],
                        start=(t == 0), stop=(t == NET - 1),
                    )
                nc.vector.tensor_copy(out=kT[:, ssl], in_=ps[:])
                # V projection, then transpose to [s, d] tiles
                xsb = pre[2] if pre and pre[2] is not None else x_chunk(xv, sc, "xv")
                ps = acc.tile([128, SC], F32, tag="acc", name="vps")
                for t in range(NET):
                    nc.tensor.matmul(
                        ps[:], lhsT=wv_sb[:, t, :], rhs=xsb[:, t, :],
                        start=(t == 0), stop=(t == NET - 1),
                    )
                vt = small.tile([128, SC], BF16, tag="vt", name="vt")
                nc.scalar.activation(out=vt[:], in_=ps[:], func=AF.Copy)
                for i in range(SC // 128):
                    tp = acc.tile([128, 128], BF16, tag="acc", name="tp")
                    nc.tensor.transpose(tp[:], vt[:, i * 128 : (i + 1) * 128], ident[:])
                    nc.vector.tensor_copy(out=v_sb[:, sc * 4 + i, :], in_=tp[:])

            def norm(qc, h):
                # l_ps already holds l broadcast on all 128 partitions
                o_ps, l_ps = state.pop((qc, h))
                qsl = slice(qc * SC, (qc + 1) * SC)
                rinv = small.tile([128, SC], F32, tag="ri", name="rinv")
                nc.vector.reciprocal_approx_fast(out=rinv[:], in_=l_ps[:])
                nc.vector.tensor_mul(onrm[:, h, qsl], o_ps[:], rinv[:])

            def attn_block(qc, extras=None):
                nkt = 4 * (qc + 1)
                nd = list(range(nkt - 4))
                # two non-diagonal tiles first (no mask-add latency in front
                # of the first exp), then the column-trimmed diagonal tiles,
                # then the rest. qc=0 has only diagonal tiles.
                order = [(kt, None) for kt in nd[:2]]
                order += [(nkt - 4 + j, j) for j in range(4)]
                order += [(kt, None) for kt in nd[2:]]
                n = len(order)
                # the PV/l flush queue is shared across the 4 heads: a head's
                # tail flushes become PE filler for the next head's early
                # iterations, keeping a 4-deep exp->PV cushion everywhere
                pend = []

                def flush_one():
                    o_ps, l_ps, kt, cl, p, first, last = pend.pop(0)
                    nc.tensor.matmul(
                        o_ps[:, cl], lhsT=v_sb[:, kt, :], rhs=p[:, cl],
                        start=first, stop=last, skip_group_check=True,
                    )
                    nc.tensor.matmul(
                        l_ps[:, cl], lhsT=ones_m[:, :], rhs=p[:, cl],
                        start=first, stop=last, skip_group_check=True,
                    )

                for h in range(G):
                    if h >= 1:
                        prev = (qc, h - 1)
                    elif qc >= 1:
                        prev = (qc - 1, 3)
                    else:
                        prev = None
                    hook = (lambda p=prev: norm(*p)) if prev is not None else None
                    quota = 4  # outproj filler rationed so every head's
                    # boundary gets PE cover, not just the first one's
                    o_ps = ops.tile([128, SC], F32, tag="o", name="o_ps")
                    l_ps = lps.tile([128, SC], F32, tag="l", name="l_ps")
                    state[(qc, h)] = (o_ps, l_ps)
                    for i, (kt, j) in enumerate(order):
                        cl = slice(128 * j, SC) if j is not None else slice(0, SC)
                        qsl = slice(qc * SC + cl.start, (qc + 1) * SC)
                        s_ps = acc.tile([128, SC], F32, tag="acc", name="s_ps")
                        nc.tensor.matmul(
                            s_ps[:, cl], lhsT=kT[:, kt * 128 : (kt + 1) * 128],
                            rhs=qT[:, h, qsl], start=True, stop=True,
                        )
                        if j is not None:
                            nc.vector.tensor_add(
                                s_ps[:, cl], s_ps[:, cl], mask_sb[:, j, cl]
                            )
                        p = pexp.tile([128, SC], BF16, tag="p", name="p")
                        nc.scalar.activation(out=p[:, cl], in_=s_ps[:, cl], func=AF.Exp)
                        pend.append((o_ps, l_ps, kt, cl, p, i == 0, i == n - 1))
                        # norm of the previous head + outproj filler go after
                        # the diagonal mask-adds so the DVE queue serves the
                        # adds (which gate exp -> PV) first
                        if i == 5 and hook is not None:
                            hook()
                            hook = None
                        if extras is not None and i >= 6 and quota > 0:
                            for _ in range(2):
                                ex = next(extras, None)
                                if ex is not None:
                                    ex()
                                quota -= 1
                        # NOTE: depth 4 is also a correctness bound: the qc=0
                        # fallback norm fires right after the next head's
                        # loop, by which point all 4 of the previous head's
                        # flushes (incl. the stop) must have been emitted.
                        while len(pend) > 4:
                            flush_one()
                    if hook is not None:
                        hook()  # loop too short (qc=0): norm after the loop
                while pend:
                    flush_one()
                if extras is not None:
                    for ex in extras:  # drain leftovers (shouldn't happen)
                        ex()

            def op_gen(qc, split_copy=False):
                # out-projection of chunk qc: 16 chains of 4 matmuls; the 4
                # e-chunks of one s-tile stage into one SBUF tile so the out
                # DMA writes full 4KB partition lines.
                for sti in range(4):
                    st = qc * 4 + sti
                    stl = slice(st * 128, (st + 1) * 128)
                    holder = {}
                    for ec in range(NEC):
                        esl = slice(ec * SC, (ec + 1) * SC)

                        def chain(stl=stl, esl=esl, st=st, ec=ec, holder=holder):
                            if ec == 0:
                                holder["ob"] = obp.tile(
                                    [128, NEC, SC], BF16, tag="ob", name="ob"
                                )
                            ps = acc.tile([128, SC], F32, tag="acc", name="ops_ps")
                            for hh in range(G):
                                nc.tensor.matmul(
                                    ps[:], lhsT=onrm[:, hh, stl], rhs=wo_sb[:, hh, esl],
                                    start=(hh == 0), stop=(hh == G - 1),
                                )
                            # at the very end of the kernel ACT is idle:
                            # alternate copies across both engines to drain
                            if split_copy and ec % 2 == 1:
                                nc.scalar.activation(
                                    out=holder["ob"][:, ec, :], in_=ps[:], func=AF.Copy
                                )
                            else:
                                nc.vector.tensor_copy(out=holder["ob"][:, ec, :], in_=ps[:])
                            if ec == NEC - 1:
                                # split over 4 queues so the last tile doesn't
                                # leave a single-queue drain tail
                                for pr in range(0, 128, 32):
                                    nc.sync.dma_start(
                                        out=out[st * 128 + pr : st * 128 + pr + 32, :],
                                        in_=holder["ob"][pr : pr + 32, :, :],
                                    )

                        yield chain

            # ---- emission schedule ----
            # interleave the first xq chunk's splits with wq's: DMA triggers
            # serialize on the sync engine (~0.6us each), so the first Q-proj
            # matmul's two dependencies must be the first two triggered.
            xq0 = xcp.tile([128, NET, SC], BF16, tag="xq", name="xq0", bufs=2)
            for i in range(8):
                t0 = i * 2
                nc.sync.dma_start(
                    out=xq0[:, t0 : t0 + 2, :], in_=xq[0:128, t0 * SC : (t0 + 2) * SC]
                )
                nc.sync.dma_start(
                    out=wq_sb[:, t0 : t0 + 2, :], in_=wq[:, t0 * GD : (t0 + 2) * GD]
                )
            # A0 sits right after P0: it only needs chunk-0 projections and
            # its 10us of DMA-free PE work covers the chunk-1 x stream.
            proj_block(0, pre=(xq0, None, None))
            attn_block(0)
            proj_block(1)
            proj_block(2)
            # wo queues behind the x chunks it must not delay; it is only
            # needed from the A1-interleaved out-projection onwards.
            wsplit(wo_sb, wo, E, 4)
            # chunk-3 x queues ahead of A1's out-DMAs
            pre3 = prefetch_x(3)
            attn_block(1, extras=op_gen(0))
            proj_block(3, pre=pre3)
            attn_block(2, extras=op_gen(1))
            attn_block(3, extras=op_gen(2))
            norm(3, 3)
            for ch in op_gen(3, split_copy=True):
                ch()
    nc.compile()
    return nc


_NC_CACHE = None


def _get_nc():
    global _NC_CACHE
    if _NC_CACHE is None:
        _NC_CACHE = build_nc()
    return _NC_CACHE


def _block_x(xT_bf):
    """[E, S] bf16 -> [sc][p][t][f] partition-major blocks [NSC*128, NET*SC]."""
    return np.ascontiguousarray(
        xT_bf.reshape(NET, 128, NSC, SC).transpose(2, 1, 0, 3).reshape(NSC * 128, NET * SC)
    )


def _block_w(w, width):
    """[ntiles*128, width] -> partition-major [128, ntiles*width]."""
    nt = w.shape[0] // 128
    return np.ascontiguousarray(
        w.reshape(nt, 128, width).transpose(1, 0, 2).reshape(128, nt * width)
    )


def _prep_in_maps(query, key, value, attn_mask, Wq, Wk, Wv, Wo):
    query = np.asarray(query, dtype=np.float32)
    key = np.asarray(key, dtype=np.float32)
    value = np.asarray(value, dtype=np.float32)
    Wq = np.asarray(Wq, dtype=np.float32)
    Wk = np.asarray(Wk, dtype=np.float32)
    Wv = np.asarray(Wv, dtype=np.float32)
    Wo = np.asarray(Wo, dtype=np.float32)
    am = np.asarray(attn_mask)

    xqT = [_block_x(np.ascontiguousarray(query[b].T).astype(NPBF)) for b in range(B)]
    xkT = [_block_x(np.ascontiguousarray(key[b].T).astype(NPBF)) for b in range(B)]
    xvT = [_block_x(np.ascontiguousarray(value[b].T).astype(NPBF)) for b in range(B)]

    # the kernel generates the causal mask on-device; sanity-check the input
    # mask really is causal (it is for this problem by construction)
    assert np.array_equal(
        np.asarray(am[0, 0, :4, :4]), np.tril(np.ones((4, 4), am.dtype))
    )

    in_maps = []
    for b in range(B):
        for g in range(HKV):
            wq_g = np.ascontiguousarray(Wq[g * GD : (g + 1) * GD, :].T).astype(NPBF)
            wk_g = np.ascontiguousarray(Wk[g * D : (g + 1) * D, :].T).astype(NPBF)
            wv_g = np.ascontiguousarray(Wv[g * D : (g + 1) * D, :].T).astype(NPBF)
            wo_g = np.ascontiguousarray(Wo[:, g * GD : (g + 1) * GD].T).astype(NPBF)
            in_maps.append(
                {
                    "xq": xqT[b],
                    "xk": xkT[b],
                    "xv": xvT[b],
                    "wq": _block_w(wq_g, GD),
                    "wk": _block_w(wk_g, D),
                    "wv": _block_w(wv_g, D),
                    "wo": _block_w(wo_g, E),
                }
            )
    return in_maps


def _run(inputs, trace=False, **kw):
    nc = _get_nc()
    in_maps = _prep_in_maps(**inputs)
    res = run_bass_kernel_spmd(nc, in_maps, list(range(NCORES)), trace=trace, **kw)
    outs = [np.asarray(r["out"]) for r in res.results]
    full = np.empty((B, S, E), dtype=np.float32)
    for b in range(B):
        acc = outs[b * HKV].astype(np.float32)
        for g in range(1, HKV):
            acc = acc + outs[b * HKV + g].astype(np.float32)
        full[b] = acc
    return full, res


def kernel(**inputs):
    full, _ = _run(inputs, trace=False)
    return full


# revision 58
# speedup vs baseline: 1.0005x; 1.0005x over previous
"""GQA kernel for Trainium2, 8 NeuronCores.  ~267us (baseline was 477us).

Sharding: core c = b*4 + g handles batch b, kv-head g (4 query heads).
Host sums the 4 partial out-projections per batch. The causal mask is
generated on-device (gpsimd affine_select); the input attn_mask is only
sanity-checked for causality.

Design notes:
- All matmuls bf16 (fp32 PSUM). PE cost is free-dim rows x clock, and the
  clock p-state ramps to 2.4GHz only after ~3us of *continuous* PE busy
  (any idle resets it), so the schedule never lets the PE starve:
  P0, A0 (covers chunk-1 DMA), P1, P2, A1+O0, P3, A2+O1, A3+O2, O3,
  with O(qc) rationed into A(qc+1)'s head loops (4 chains/head at
  iterations 6-7) and ~80 warmup matmuls bridging the initial DMA wait.
- DMA descriptors are per SBUF partition line (~43-130ns each regardless
  of size), so all DRAM layouts are partition-major: x is host-blocked
  [sc][p][t][f] (4KB lines), weights [p][t][..], output per s-tile as
  full [128, E] rows (4KB lines, split over 4 queues). Chunk loads split
  along t so queues stream in parallel and a chain's first matmul only
  waits for its own t-range. wq/xq0 split-triggers interleave because
  DMA triggers serialize on the sync engine.
- Projections run Q-first per chunk: the 13.6us Q-proj paces the
  chunk's K/V x stream; xq double-buffers so the next chunk's stream is
  not serialized behind the previous Q-proj (WAR).
- Attention kt order: two non-diagonal tiles first (no DVE mask-add in
  front of the first exps), then the 4 diagonal tiles (column-trimmed:
  tile j only touches q columns >= 128j, the rest is fully masked), then
  the remainder. PV/l flushes trail by 4 iterations in a queue shared
  across the 4 heads, so a head's tail flushes fill the next head's
  start. The first flushed PV/l matmul covers all 512 columns so PSUM
  start=True initializes the full accumulator.
- l row-sums accumulate via [128,128]-ones matmuls (same row cost as a
  1-wide lhsT but the weight load pipelines, and the result lands
  already broadcast across partitions); the norm is then just
  reciprocal_approx_fast (DVE, ~0.7us vs 3.3us for reciprocal) straight
  off PSUM and one multiply. The previous head's norm is emitted inside
  the next head's loop after the diagonal mask-adds so the DVE serves
  the adds (which gate exp -> PV) first.
"""

import sys

import numpy as np

for _p in ("/opt/trn_rl_repo",):
    if _p not in sys.path:
        sys.path.insert(0, _p)

import ml_dtypes

import concourse.mybir as mybir
from concourse import bacc
from concourse.bass_utils import run_bass_kernel_spmd
from concourse.masks import make_identity
from concourse.tile import TileContext

B, S, E = 2, 2048, 2048
H, HKV = 16, 4
D = E // H  # 128
G = H // HKV  # 4 query heads per kv head
GD = G * D  # 512
NCORES = B * HKV  # 8
SC = 512  # s/q chunk width (free dim of matmuls)
NSC = S // SC  # 4
NET = E // 128  # 16 e-tiles (contraction)
NKT = S // 128  # 16 k-tiles
NEC = E // SC  # 4 e-chunks for output
SCALE = 1.0 / float(np.sqrt(D))

F32 = mybir.dt.float32
BF16 = mybir.dt.bfloat16
AF = mybir.ActivationFunctionType
NPBF = np.dtype(ml_dtypes.bfloat16)


def build_nc():
    nc = bacc.Bacc()
    # x inputs: [sc][p][t][f] partition-major blocks -> [NSC*128, NET*SC]
    xq = nc.declare_dram_parameter("xq", [NSC * 128, NET * SC], BF16, isOutput=False)
    xk = nc.declare_dram_parameter("xk", [NSC * 128, NET * SC], BF16, isOutput=False)
    xv = nc.declare_dram_parameter("xv", [NSC * 128, NET * SC], BF16, isOutput=False)
    # weights partition-major: [p][t][..]
    wq = nc.declare_dram_parameter("wq", [128, NET * GD], BF16, isOutput=False)
    wk = nc.declare_dram_parameter("wk", [128, NET * D], BF16, isOutput=False)
    wv = nc.declare_dram_parameter("wv", [128, NET * D], BF16, isOutput=False)
    wo = nc.declare_dram_parameter("wo", [128, G * E], BF16, isOutput=False)
    # natural [S, E]: written per s-tile as [128, E] full-width rows
    out = nc.declare_dram_parameter("out", [S, E], BF16, isOutput=True)

    with TileContext(nc) as tc:
        with (
            tc.tile_pool(name="singles", bufs=1) as singles,
            tc.tile_pool(name="xc", bufs=1) as xcp,
            tc.tile_pool(name="pexp", bufs=7) as pexp,
            tc.tile_pool(name="small", bufs=2) as small,
            tc.tile_pool(name="ob", bufs=2) as obp,
            tc.tile_pool(name="acc", bufs=4, space="PSUM") as acc,
            tc.tile_pool(name="ops", bufs=2, space="PSUM") as ops,
            tc.tile_pool(name="lps", bufs=2, space="PSUM") as lps,
        ):
            # ---- constants / weights resident in SBUF ----
            wq_sb = singles.tile([128, NET, GD], BF16)  # 16KB/p
            wk_sb = singles.tile([128, NET, D], BF16)  # 4KB/p
            wv_sb = singles.tile([128, NET, D], BF16)  # 4KB/p
            wo_sb = singles.tile([128, G, E], BF16)  # 16KB/p
            mask_sb = singles.tile([128, 4, SC], F32)  # 8KB/p
            ident_f = singles.tile([128, 128], F32)
            ident = singles.tile([128, 128], BF16)
            ones_f = singles.tile([128, 128], F32)
            # l-matmul lhsT: [128,128] ones -> row-sum REPLICATED on all 128
            # output partitions (same row cost as a 1-wide lhsT, but the
            # weight load pipelines and no separate broadcast is needed)
            ones_m = singles.tile([128, 128], BF16)
            qT = singles.tile([128, G, S], BF16)  # 16KB/p
            kT = singles.tile([128, S], BF16)  # 4KB/p
            v_sb = singles.tile([128, NKT, D], BF16)  # 4KB/p
            onrm = singles.tile([128, G, S], BF16)  # 16KB/p

            make_identity(nc, ident_f)
            nc.scalar.activation(out=ident[:], in_=ident_f[:], func=AF.Copy)
            nc.vector.memset(ones_f, 1.0)
            nc.scalar.activation(out=ones_m[:], in_=ones_f[:], func=AF.Copy)
            # causal mask built on-device (gpsimd, idle at t=0): tile j is the
            # additive mask for k-rows 128j..128j+127 vs q-columns of a chunk:
            # keep 0 where q - p - 128j >= 0, else -1e9.
            nc.gpsimd.memset(mask_sb, 0.0)
            for j in range(4):
                nc.gpsimd.affine_select(
                    out=mask_sb[:, j, :], in_=mask_sb[:, j, :],
                    compare_op=mybir.AluOpType.is_ge, fill=-1e9,
                    base=-128 * j, channel_multiplier=-1, pattern=[[1, SC]],
                )

            def wsplit(dst, dram, width, nsplit):
                """Load [128, n, width] SBUF tile from partition-major DRAM."""
                ntiles = dst.shape[1]
                step = ntiles // nsplit
                for i in range(nsplit):
                    t0 = i * step
                    nc.sync.dma_start(
                        out=dst[:, t0 : t0 + step, :],
                        in_=dram[:, t0 * width : (t0 + step) * width],
                    )

            # PE p-state warmup: the clock ramps to 2.4GHz only after ~3us of
            # continuous busy, and the first x split lands at ~14us. These
            # dummy matmuls bridge the idle window so real work starts at
            # full clock.
            warm_ps = acc.tile([128, SC], F32, tag="acc", name="warm_ps")

            def pad(k):
                """k dummy matmuls: PE filler that holds the p-state clock up
                across a known dependency stall (idle resets the 2.4GHz ramp)."""
                for _ in range(k):
                    nc.tensor.matmul(
                        warm_ps[:, 0:128], lhsT=ident[:], rhs=ident[:],
                        start=True, stop=True, skip_group_check=True,
                    )

            pad(80)

            state = {}  # (qc, h) -> (o_ps, l_ps)

            def x_chunk(dram, sc, tag):
                """Stream one [128, NET, SC] x chunk, split along t."""
                # xq double-buffers: its ring-1 WAR (next chunk's DMA vs the
                # 4 head chains of the previous Q-proj) otherwise serializes
                # the stream late in the run.
                xsb = xcp.tile(
                    [128, NET, SC], BF16, tag=tag, name=tag,
                    bufs=2 if tag == "xq" else 1,
                )
                rows = slice(sc * 128, (sc + 1) * 128)
                # 4KB descriptor lines in steady state; finer first chunk so
                # the first projection chain starts earlier
                nsplit = 8 if sc == 0 else 4
                step = NET // nsplit
                for i in range(nsplit):
                    t0 = i * step
                    nc.sync.dma_start(
                        out=xsb[:, t0 : t0 + step, :],
                        in_=dram[rows, t0 * SC : (t0 + step) * SC],
                    )
                return xsb

            def prefetch_x(sc):
                return (
                    x_chunk(xq, sc, "xq"),
                    x_chunk(xk, sc, "xk"),
                    x_chunk(xv, sc, "xv"),
                )

            def proj_block(sc, pre=None):
                # Q first: its 13.6us of chains pace the chunk's K/V x stream
                ssl = slice(sc * SC, (sc + 1) * SC)
                xsb = pre[0] if pre and pre[0] is not None else x_chunk(xq, sc, "xq")
                for h in range(G):
                    ps = acc.tile([128, SC], F32, tag="acc", name="qps")
                    for t in range(NET):
                        nc.tensor.matmul(
                            ps[:], lhsT=wq_sb[:, t, h * D : (h + 1) * D],
                            rhs=xsb[:, t, :], start=(t == 0), stop=(t == NET - 1),
                        )
                    # fold softmax scale into Q
                    nc.scalar.activation(
                        out=qT[:, h, ssl], in_=ps[:], func=AF.Copy, scale=SCALE
                    )
                    if sc == 0 and h == 0:
                        wsplit(wk_sb, wk, D, 4)
                        wsplit(wv_sb, wv, D, 4)
                # K projection
                xsb = pre[1] if pre and pre[1] is not None else x_chunk(xk, sc, "xk")
                ps = acc.tile([128, SC], F32, tag="acc", name="kps")
                for t in range(NET):
                    nc.tensor.matmul(
                        ps[:], lhsT=wk_sb[:, t, :], rhs=xsb[:, t, :],
                        start=(t == 0), stop=(t == NET - 1),
                    )
                nc.vector.tensor_copy(out=kT[:, ssl], in_=ps[:])
                # V projection, then transpose to [s, d] tiles
                xsb = pre[2] if pre and pre[2] is not None else x_chunk(xv, sc, "xv")
                ps = acc.tile([128, SC], F32, tag="acc", name="vps")
                for t in range(NET):
                    nc.tensor.matmul(
                        ps[:], lhsT=wv_sb[:, t, :], rhs=xsb[:, t, :],
                        start=(t == 0), stop=(t == NET - 1),
                    )
                vt = small.tile([128, SC], BF16, tag="vt", name="vt")
                nc.scalar.activation(out=vt[:], in_=ps[:], func=AF.Copy)
                for i in range(SC // 128):
                    tp = acc.tile([128, 128], BF16, tag="acc", name="tp")
                    nc.tensor.transpose(tp[:], vt[:, i * 128 : (i + 1) * 128], ident[:])
                    nc.vector.tensor_copy(out=v_sb[:, sc * 4 + i, :], in_=tp[:])

            def norm(qc, h):
                # l_ps already holds l broadcast on all 128 partitions
                o_ps, l_ps = state.pop((qc, h))
                qsl = slice(qc * SC, (qc + 1) * SC)
                rinv = small.tile([128, SC], F32, tag="ri", name="rinv")
                nc.vector.reciprocal_approx_fast(out=rinv[:], in_=l_ps[:])
                nc.vector.tensor_mul(onrm[:, h, qsl], o_ps[:], rinv[:])

            def attn_block(qc, extras=None):
                nkt = 4 * (qc + 1)
                nd = list(range(nkt - 4))
                # two non-diagonal tiles first (no mask-add latency in front
                # of the first exp), then the column-trimmed diagonal tiles,
                # then the rest. qc=0 has only diagonal tiles.
                order = [(kt, None) for kt in nd[:2]]
                order += [(nkt - 4 + j, j) for j in range(4)]
                order += [(kt, None) for kt in nd[2:]]
                n = len(order)
                # the PV/l flush queue is shared across the 4 heads: a head's
                # tail flushes become PE filler for the next head's early
                # iterations, keeping a 4-deep exp->PV cushion everywhere
                pend = []

                def flush_one():
                    o_ps, l_ps, kt, cl, p, first, last = pend.pop(0)
                    nc.tensor.matmul(
                        o_ps[:, cl], lhsT=v_sb[:, kt, :], rhs=p[:, cl],
                        start=first, stop=last, skip_group_check=True,
                    )
                    nc.tensor.matmul(
                        l_ps[:, cl], lhsT=ones_m[:, :], rhs=p[:, cl],
                        start=first, stop=last, skip_group_check=True,
                    )

                for h in range(G):
                    if h >= 1:
                        prev = (qc, h - 1)
                    elif qc >= 1:
                        prev = (qc - 1, 3)
                    else:
                        prev = None
                    hook = (lambda p=prev: norm(*p)) if prev is not None else None
                    quota = 4  # outproj filler rationed so every head's
                    # boundary gets PE cover, not just the first one's
                    o_ps = ops.tile([128, SC], F32, tag="o", name="o_ps")
                    l_ps = lps.tile([128, SC], F32, tag="l", name="l_ps")
                    state[(qc, h)] = (o_ps, l_ps)
                    for i, (kt, j) in enumerate(order):
                        cl = slice(128 * j, SC) if j is not None else slice(0, SC)
                        qsl = slice(qc * SC + cl.start, (qc + 1) * SC)
                        s_ps = acc.tile([128, SC], F32, tag="acc", name="s_ps")
                        nc.tensor.matmul(
                            s_ps[:, cl], lhsT=kT[:, kt * 128 : (kt + 1) * 128],
                            rhs=qT[:, h, qsl], start=True, stop=True,
                        )
                        if j is not None:
                            nc.vector.tensor_add(
                                s_ps[:, cl], s_ps[:, cl], mask_sb[:, j, cl]
                            )
                        p = pexp.tile([128, SC], BF16, tag="p", name="p")
                        nc.scalar.activation(out=p[:, cl], in_=s_ps[:, cl], func=AF.Exp)
                        pend.append((o_ps, l_ps, kt, cl, p, i == 0, i == n - 1))
                        # norm of the previous head + outproj filler go after
                        # the diagonal mask-adds so the DVE queue serves the
                        # adds (which gate exp -> PV) first
                        if i == 5 and hook is not None:
                            hook()
                            hook = None
                        if extras is not None and i >= 6 and quota > 0:
                            for _ in range(2):
                                ex = next(extras, None)
                                if ex is not None:
                                    ex()
                                quota -= 1
                        # NOTE: depth 4 is also a correctness bound: the qc=0
                        # fallback norm fires right after the next head's
                        # loop, by which point all 4 of the previous head's
                        # flushes (incl. the stop) must have been emitted.
                        while len(pend) > 4:
                            flush_one()
                    if hook is not None:
                        hook()  # loop too short (qc=0): norm after the loop
                while pend:
                    flush_one()
                if extras is not None:
                    for ex in extras:  # drain leftovers (shouldn't happen)
                        ex()

            def op_gen(qc, split_copy=False):
                # out-projection of chunk qc: 16 chains of 4 matmuls; the 4
                # e-chunks of one s-tile stage into one SBUF tile so the out
                # DMA writes full 4KB partition lines.
                for sti in range(4):
                    st = qc * 4 + sti
                    stl = slice(st * 128, (st + 1) * 128)
                    holder = {}
                    for ec in range(NEC):
                        esl = slice(ec * SC, (ec + 1) * SC)

                        def chain(stl=stl, esl=esl, st=st, ec=ec, holder=holder):
                            if ec == 0:
                                holder["ob"] = obp.tile(
                                    [128, NEC, SC], BF16, tag="ob", name="ob"
                                )
                            ps = acc.tile([128, SC], F32, tag="acc", name="ops_ps")
                            for hh in range(G):
                                nc.tensor.matmul(
                                    ps[:], lhsT=onrm[:, hh, stl], rhs=wo_sb[:, hh, esl],
                                    start=(hh == 0), stop=(hh == G - 1),
                                )
                            # at the very end of the kernel ACT is idle:
                            # alternate copies across both engines to drain
                            if split_copy and ec % 2 == 1:
                                nc.scalar.activation(
                                    out=holder["ob"][:, ec, :], in_=ps[:], func=AF.Copy
                                )
                            else:
                                nc.vector.tensor_copy(out=holder["ob"][:, ec, :], in_=ps[:])
                            if ec == NEC - 1:
                                # split over 4 queues so the last tile doesn't
                                # leave a single-queue drain tail
                                for pr in range(0, 128, 32):
                                    nc.sync.dma_start(
                                        out=out[st * 128 + pr : st * 128 + pr + 32, :],
                                        in_=holder["ob"][pr : pr + 32, :, :],
                                    )

                        yield chain

            # ---- emission schedule ----
            # interleave the first xq chunk's splits with wq's: DMA triggers
            # serialize on the sync engine (~0.6us each), so the first Q-proj
            # matmul's two dependencies must be the first two triggered.
            xq0 = xcp.tile([128, NET, SC], BF16, tag="xq", name="xq0", bufs=2)
            for i in range(8):
                t0 = i * 2
                nc.sync.dma_start(
                    out=xq0[:, t0 : t0 + 2, :], in_=xq[0:128, t0 * SC : (t0 + 2) * SC]
                )
                nc.sync.dma_start(
                    out=wq_sb[:, t0 : t0 + 2, :], in_=wq[:, t0 * GD : (t0 + 2) * GD]
                )
            # A0 sits right after P0: it only needs chunk-0 projections and
            # its 10us of DMA-free PE work covers the chunk-1 x stream.
            proj_block(0, pre=(xq0, None, None))
            attn_block(0)
            proj_block(1)
            proj_block(2)
            # wo queues behind the x chunks it must not delay; it is only
            # needed from the A1-interleaved out-projection onwards.
            wsplit(wo_sb, wo, E, 4)
            # chunk-3 x queues ahead of A1's out-DMAs
            pre3 = prefetch_x(3)
            attn_block(1, extras=op_gen(0))
            proj_block(3, pre=pre3)
            attn_block(2, extras=op_gen(1))
            attn_block(3, extras=op_gen(2))
            norm(3, 3)
            for ch in op_gen(3, split_copy=True):
                ch()
    nc.compile()
    return nc


_NC_CACHE = None


def _get_nc():
    global _NC_CACHE
    if _NC_CACHE is None:
        _NC_CACHE = build_nc()
    return _NC_CACHE


def _block_x(xT_bf):
    """[E, S] bf16 -> [sc][p][t][f] partition-major blocks [NSC*128, NET*SC]."""
    return np.ascontiguousarray(
        xT_bf.reshape(NET, 128, NSC, SC).transpose(2, 1, 0, 3).reshape(NSC * 128, NET * SC)
    )


def _block_w(w, width):
    """[ntiles*128, width] -> partition-major [128, ntiles*width]."""
    nt = w.shape[0] // 128
    return np.ascontiguousarray(
        w.reshape(nt, 128, width).transpose(1, 0, 2).reshape(128, nt * width)
    )


def _prep_in_maps(query, key, value, attn_mask, Wq, Wk, Wv, Wo):
    query = np.asarray(query, dtype=np.float32)
    key = np.asarray(key, dtype=np.float32)
    value = np.asarray(value, dtype=np.float32)
    Wq = np.asarray(Wq, dtype=np.float32)
    Wk = np.asarray(Wk, dtype=np.float32)
    Wv = np.asarray(Wv, dtype=np.float32)
    Wo = np.asarray(Wo, dtype=np.float32)
    am = np.asarray(attn_mask)

    xqT = [_block_x(np.ascontiguousarray(query[b].T).astype(NPBF)) for b in range(B)]
    xkT = [_block_x(np.ascontiguousarray(key[b].T).astype(NPBF)) for b in range(B)]
    xvT = [_block_x(np.ascontiguousarray(value[b].T).astype(NPBF)) for b in range(B)]

    # the kernel generates the causal mask on-device; sanity-check the input
    # mask really is causal (it is for this problem by construction)
    assert np.array_equal(
        np.asarray(am[0, 0, :4, :4]), np.tril(np.ones((4, 4), am.dtype))
    )

    in_maps = []
    for b in range(B):
        for g in range(HKV):
            wq_g = np.ascontiguousarray(Wq[g * GD : (g + 1) * GD, :].T).astype(NPBF)
            wk_g = np.ascontiguousarray(Wk[g * D : (g + 1) * D, :].T).astype(NPBF)
            wv_g = np.ascontiguousarray(Wv[g * D : (g + 1) * D, :].T).astype(NPBF)
            wo_g = np.ascontiguousarray(Wo[:, g * GD : (g + 1) * GD].T).astype(NPBF)
            in_maps.append(
                {
                    "xq": xqT[b],
                    "xk": xkT[b],
                    "xv": xvT[b],
                    "wq": _block_w(wq_g, GD),
                    "wk": _block_w(wk_g, D),
                    "wv": _block_w(wv_g, D),
                    "wo": _block_w(wo_g, E),
                }
            )
    return in_maps


def _run(inputs, trace=False, **kw):
    nc = _get_nc()
    in_maps = _prep_in_maps(**inputs)
    res = run_bass_kernel_spmd(nc, in_maps, list(range(NCORES)), trace=trace, **kw)
    outs = [np.asarray(r["out"]) for r in res.results]
    full = np.empty((B, S, E), dtype=np.float32)
    for b in range(B):
        acc = outs[b * HKV].astype(np.float32)
        for g in range(1, HKV):
            acc = acc + outs[b * HKV + g].astype(np.float32)
        full[b] = acc
    return full, res


def kernel(**inputs):
    full, _ = _run(inputs, trace=False)
    return full
